# revision 40
# baseline (speedup 1.0000x reference)
"""BEVFormer encoder layer on 8 Trainium2 NeuronCores (Bass/Tile, SPMD).

Sharding: data-parallel over the 4096 BEV queries (512 per core); K/V and the
deformable value table are computed redundantly per core, so no collectives.
Large matmuls run in bf16 (fp32 is 4x slower on the PE); the residual stream,
layernorms, softmax statistics and sampling arithmetic stay fp32. Deformable
sampling uses a DRAM "quad" table (each row = a 2x2 pixel neighborhood of one
head, padded ring; quad-major rows so all 8 heads of a quad are contiguous)
gathered with native indirect DMA (32 calls per level x query-chunk, 128
offsets each, spread over 4 SWDGE queues). Flash self-attention accumulates
attn@V and the softmax denominator directly in PSUM across key chunks.
"""
import numpy as np
import ml_dtypes
from contextlib import ExitStack

import concourse.bass as bass
import concourse.tile as tile
from concourse import mybir
from concourse.bass import IndirectOffsetOnAxis

from concourse.bass_utils import run_bass_kernel_spmd

# ---------------------------------------------------------------------------
# Inlined toolchain workarounds (kernel.py must be self-contained).
# (1) NTFF profile hook for trace=True under axon (stub antenv lacks it).
# (2) walrus in this image allows only ONE sem-wait per instruction; Tile
#     emits multi-wait instructions, so hoist extras onto same-engine nops.
# ---------------------------------------------------------------------------
import contextlib as _ctxlib
import ctypes as _ctypes
import sys as _sys
import types as _types

def _install_ntff_hook():
    so_path = "/opt/axon/libaxon_pjrt.so"
    try:
        lib = _ctypes.CDLL(so_path)
    except OSError:
        lib = None
    if lib is None or not hasattr(lib, "axon_start_nrt_profile"):
        hook = None
    else:
        lib.axon_start_nrt_profile.argtypes = [_ctypes.POINTER(_ctypes.c_int64), _ctypes.c_size_t]
        lib.axon_start_nrt_profile.restype = _ctypes.c_int64
        lib.axon_stop_nrt_profile.argtypes = [_ctypes.c_char_p]
        lib.axon_stop_nrt_profile.restype = _ctypes.c_int64

        @_ctxlib.contextmanager
        def hook(output_dir, device_ids):
            import jax
            jax.devices()
            if device_ids:
                ids = (_ctypes.c_int64 * len(device_ids))(*device_ids)
                rc = lib.axon_start_nrt_profile(ids, len(device_ids))
            else:
                rc = lib.axon_start_nrt_profile(None, 0)
            if rc != 0:
                raise RuntimeError(f"axon_start_nrt_profile rc={rc}")
            try:
                yield
            finally:
                n = lib.axon_stop_nrt_profile(str(output_dir).encode())
                if n < 0:
                    raise RuntimeError(f"axon_stop_nrt_profile rc={n}")

    mod = _types.ModuleType("antenv.axon_hooks")
    mod.get_axon_ntff_profile_hook = lambda: hook
    mod.set_axon_ntff_profile_hook = lambda h: None
    _sys.modules["antenv.axon_hooks"] = mod

_install_ntff_hook()

from concourse.tile import ScopedClock as _ScopedClock

def _patched_drain_and_barrier(self, tick_clock, wait_clock):
    nc_ = self.nc
    drain_inst = nc_.sync.drain()
    inst = drain_inst.ins
    wait_clock.add_sem_waits(inst, _ScopedClock({None: tick_clock.global_clock}))
    si = inst.sync_info
    waits = list(si.on_wait or []) if si else []
    if len(waits) > 1:
        si.on_wait = waits[:1]
        inst.sync_info = si
        for w in waits[1:]:
            nop = nc_.sync.nop()
            ni = nop.ins
            ni.sync_info = mybir.SyncInfo(on_wait=[w], on_update=[])
    nc_.all_engine_barrier()
    assert self.sems is not None
    popped = nc_._tile_sem_poison_stack.pop()
    assert popped is self._sem_poison
    nc_.clear_and_free_semaphores(list(self.sems.allocated().values()))
    nc_.all_engine_barrier()

tile.TileContext._drain_and_barrier = _patched_drain_and_barrier

def _split_multi_waits(nc_, max_waits=1):
    n_split = 0
    for f in nc_.m.functions:
        for bb in f.blocks:
            out = []
            changed = False
            for inst in bb.instructions:
                si = inst.sync_info
                waits = list(si.on_wait) if si and si.on_wait else []
                if len(waits) > max_waits:
                    reg_waits = [w for w in waits if w.wait_reg is not None]
                    imm_waits = [w for w in waits if w.wait_reg is None]
                    keep_n = max(0, max_waits - len(reg_waits))
                    nh = len(imm_waits) - keep_n if keep_n < len(imm_waits) else 0
                    hoist = imm_waits[:nh]
                    if hoist:
                        changed = True
                        for w in hoist:
                            n_split += 1
                            nop = mybir.InstNoOp(name=f"waitsplit-{n_split}")
                            nop.engine = inst.engine
                            nop.sync_info = mybir.SyncInfo(on_wait=[w], on_update=[])
                            out.append(nop)
                        si.on_wait = reg_waits + imm_waits[nh:]
                        inst.sync_info = si
                out.append(inst)
            if changed:
                bb.instructions = out
    return n_split


F32 = mybir.dt.float32
BF16 = mybir.dt.bfloat16
I32 = mybir.dt.int32
AF = mybir.ActivationFunctionType
ALU = mybir.AluOpType

D = 256
H = 8
LQ = 4096
LQC = 512          # queries per core
NQC = LQC // 128   # q-chunks per core
LV = 5376
DFFN = 512
SHAPES = [(64, 64), (32, 32), (16, 16)]
LVL_CHUNKS = [(0, 32), (32, 40), (40, 42)]
W2S = [w + 2 for _, w in SHAPES]
R_L = [(h + 2) * (w + 2) for h, w in SHAPES]
QB_L = [0, 8 * R_L[0], 8 * (R_L[0] + R_L[1])]
QT_ROWS = 8 * sum(R_L)
NKC = 32
EPS = 1e-5
BATCH_GATHER = False


def build_kernel(split_waits=True):
    nc = bass.Bass("TRN2", target_bir_lowering=False, debug=False, num_devices=8,
                   num_swdge_queues=4)

    din = {}
    def dI(name, shape, dt=F32):
        din[name] = nc.dram_tensor(name, shape, dt, kind="ExternalInput").ap()

    dI("bqT_bf", [D, LQ], BF16)
    dI("posT_bf", [D, LQ], BF16)
    dI("imgT_bf", [D, LV], BF16)
    dI("bqcT", [D, LQC], F32)
    dI("poscT", [D, LQC], F32)
    dI("refq", [128, NQC, 6], F32)
    dI("wqT", [D, D], BF16)
    dI("wkT", [D, D], BF16)
    dI("wvT", [D, D], BF16)
    dI("woT", [D, D], BF16)
    dI("offwT", [D, 192], BF16)
    dI("offb", [1, 192], F32)
    dI("attnwT", [D, 96], BF16)
    dI("valwT", [D, D], BF16)
    dI("cawT", [D, D], BF16)
    dI("w1T", [D, DFFN], BF16)
    dI("w2T", [DFFN, D], BF16)
    dI("lnp", [128, 2, 6], F32)
    dI("pshift", [128, 14 * 128], BF16)
    dI("cb0", [128, 32], F32)
    dI("cb1", [128, 32], F32)
    dI("cb2", [128, 32], F32)
    dI("idt", [128, 128], BF16)

    outT = nc.dram_tensor("outT", [D, LQC], F32, kind="ExternalOutput").ap()
    # one quad table per level: gathers of level l then only depend on that
    # level's writes (Tile DRAM deps are per-tensor)
    qt_l = [nc.dram_tensor(f"qt{l}", [8 * R_L[l], 128], BF16) for l in range(3)]
    qta_l = [t.ap() for t in qt_l]

    BIG = ("bqT_bf", "posT_bf", "imgT_bf")
    with tile.TileContext(nc) as tc, ExitStack() as ctx:
        cst = ctx.enter_context(tc.tile_pool(name="cst", bufs=1))
        res = ctx.enter_context(tc.tile_pool(name="res", bufs=1))
        ebp = ctx.enter_context(tc.tile_pool(name="ebp", bufs=2))
        pinV = ctx.enter_context(tc.tile_pool(name="pinV", bufs=1))
        vq = ctx.enter_context(tc.tile_pool(name="vq", bufs=1))
        # flash-only tensors: freed after the flash finalize so the
        # deformable pools (gq double-buffer etc.) fit in SBUF.
        # opened BEFORE pin so pool releases stay LIFO (pin closes first).
        fls_cm = tc.tile_pool(name="fls", bufs=1)
        fls = fls_cm.__enter__()
        pin_cm = tc.tile_pool(name="pin", bufs=1)
        pin = pin_cm.__enter__()
        psP_cm = tc.tile_pool(name="psP", bufs=2, space="PSUM")
        psP = psP_cm.__enter__()

        # ---------------- load inputs ----------------
        t_in = {}
        for name, ap in din.items():
            shp = list(ap.shape)
            pool_ = pin if name in BIG else cst
            if shp[0] > 128:
                tl = pool_.tile([128, shp[0] // 128, shp[1]], ap.dtype, tag=name)
                nc.sync.dma_start(tl[:], ap.rearrange("(c p) n -> p c n", p=128))
            else:
                tl = pool_.tile(shp, ap.dtype, tag=name)
                nc.sync.dma_start(tl[:], ap[:])
            t_in[name] = tl

        # this core's q block (f32 add then bf16)
        qcT = fls.tile([128, 2, LQC], BF16, tag="qcT")
        for c in range(2):
            nc.vector.tensor_add(qcT[:, c], t_in["bqcT"][:, c], t_in["poscT"][:, c])

        # ---------------- qpT [256, 512] bf16 ----------------
        qpT = fls.tile([128, 2, LQC], BF16, tag="qpT")
        for m in range(2):
            pq = psP.tile([128, 512], F32, tag="ps512")
            for k in range(2):
                nc.tensor.matmul(pq[:], t_in["wqT"][:, k, m * 128:(m + 1) * 128],
                                 qcT[:, k], start=(k == 0), stop=(k == 1))
            nc.vector.tensor_copy(qpT[:, m], pq[:])

        # ---------------- quad table ----------------
        # Zero-fill (ring borders must read as 0.0); interior rows are
        # overwritten below.
        zt = pinV.tile([128, 512], BF16, tag="zt")
        nc.gpsimd.memset(zt[:], 0.0)
        for l in range(3):
            zflat = qta_l[l].rearrange("r c -> (r c)")
            total = 8 * R_L[l] * 128
            step = 128 * 512
            o = 0
            while o < total:
                n = min(step, total - o)
                if n % 128 == 0:
                    nc.sync.dma_start(zflat[o:o + n].rearrange("(p f) -> p f", p=128),
                                      zt[:, 0:n // 128])
                else:
                    nc.sync.dma_start(zflat[o:o + n].rearrange("(p f) -> p f", p=1),
                                      zt[0:1, 0:n])
                o += n

        # Quad-table interior: build per-slice corner-interleaved tiles
        # vqh[p, c, h, 4crn, 32] in SBUF, then write the table with
        # 256B-contiguous runs (4x fewer/4x bigger HBM descriptors than
        # scattering 64B corner blocks). The partition shifts (+1, +wl, +wl+1
        # pixels) run on the PE as permutation matmuls (pshift[:, 2i]/[:, 2i+1]
        # = main/wrap shifted identities for shift SHV[i]). Deferred until
        # after flash (level 2 first) so it stays off the critical chain:
        # gathers run level 2 -> 1 -> 0 while level 0's table finishes.
        SHV = [1, 16, 17, 32, 33, 64, 65]
        psh = t_in["pshift"][:].rearrange("p (s c) -> p s c", c=128)

        def emit_build(l, pspool):
            hl, wl = SHAPES[l]
            c0, c1 = LVL_CHUNKS[l]
            w2 = W2S[l]
            ngrp = 128 // wl
            nsl = (c1 - c0 + 7) // 8
            for s in range(nsl):
                sc0 = c0 + s * 8
                scn = min(8, c1 - sc0)
                vqh = vq.tile([128, 8, 8, 4, 32], BF16, tag="vqh")
                nc.scalar.copy(
                    vqh[:, 0:scn, :, 0],
                    val[:, sc0:sc0 + scn].rearrange("p c (h e) -> p c h e", h=8))
                for dy in range(2):
                    for dx in range(2):
                        sh = dy * wl + dx
                        if sh == 0:
                            continue
                        si = SHV.index(sh)
                        for brel in range(0, scn, 2):
                            cA = sc0 + brel
                            pp = pspool.tile([128, 512], F32, tag="pss")
                            rhs_m = val[:, cA:cA + 2].rearrange("p c e -> p (c e)")
                            rhs_w = val[:, cA + 1:cA + 3].rearrange("p c e -> p (c e)")
                            nc.tensor.matmul(pp[:], psh[:, 2 * si], rhs_m,
                                             start=True, stop=False)
                            nc.tensor.matmul(pp[:], psh[:, 2 * si + 1], rhs_w,
                                             start=False, stop=True)
                            nc.scalar.copy(
                                vqh[:, brel:brel + 2, :, 2 * dy + dx],
                                pp[:].rearrange("p (c h e) -> p c h e", c=2, h=8))
                for g in range(ngrp):
                    # quad-major table rows (row = quad*8 + h): all 8 heads
                    # are 1024 contiguous elems -> one 3-dim DMA per group
                    srcap = vqh[g * wl:(g + 1) * wl, 0:scn, :]
                    y0 = (sc0 - c0) * ngrp + g
                    base = ((y0 + 1) * w2 + 1) * 8 * 128
                    dst = bass.AP(
                        qt_l[l], base,
                        [[8 * 128, wl],
                         [ngrp * w2 * 8 * 128, scn],
                         [1, 8 * 128]])
                    nc.sync.dma_start(dst, srcap)

        psP_cm.__exit__(None, None, None)
        pin_cm.__exit__(None, None, None)

        # ---------------- flash self-attention ----------------
        # attn@V and the softmax denominator accumulate together in PSUM
        # across the 32 key chunks: per head the av lhsT is 33 wide (32
        # value channels + a ones column), so oasc_g2 row 32/96 ends up
        # holding sum(exp) for the pair's two heads (64-col bands 0/64).
        psacc_cm = tc.tile_pool(name="psacc", bufs=1, space="PSUM")
        psacc = psacc_cm.__enter__()
        oasc0 = psacc.tile([128, LQC], F32, tag="oasc0")
        oasc1 = psacc.tile([128, LQC], F32, tag="oasc1")
        oasc2 = psacc.tile([128, LQC], F32, tag="oasc2")
        oasc3 = psacc.tile([128, LQC], F32, tag="oasc3")
        oasc = [oasc0, oasc1, oasc2, oasc3]
        psA_cm = tc.tile_pool(name="psA", bufs=2, space="PSUM")
        psA = psA_cm.__enter__()

        # flash chases the kpT/vp chunk computation: block n's K/V tiles are
        # emitted right before its 4 key chunks, staged through the psA
        # "pscore" slots, so the first exp fires as soon as the inputs land.
        kpT_n = [None] * 8
        vp_t = [None] * NKC
        for n in range(8):
            kt = fls.tile([128, 2, 512], BF16, tag=f"kpT{n}")
            kpT_n[n] = kt
            for m in range(2):
                pk = psA.tile([128, 512], F32, tag="pscore")
                for ki, srcn in enumerate(("bqT_bf", "posT_bf")):
                    for k in range(2):
                        nc.tensor.matmul(pk[:], t_in["wkT"][:, k, m * 128:(m + 1) * 128],
                                         t_in[srcn][:, k, n * 512:(n + 1) * 512],
                                         start=(ki == 0 and k == 0),
                                         stop=(ki == 1 and k == 1))
                nc.vector.tensor_copy(kt[:, m], pk[:])
            for ck in range(4 * n, 4 * n + 4):
                vt = fls.tile([128, 8, 33], BF16, tag=f"vp{ck}")
                vp_t[ck] = vt
                nc.gpsimd.memset(vt[:, :, 32:33], 1.0)
                pv = psA.tile([128, 512], F32, tag="pscore")
                for k in range(2):
                    nc.tensor.matmul(pv[:, 0:256],
                                     t_in["bqT_bf"][:, k, ck * 128:(ck + 1) * 128],
                                     t_in["wvT"][:, k], start=(k == 0), stop=(k == 1))
                nc.vector.tensor_copy(vt[:, :, 0:32],
                                      pv[:, 0:256].rearrange("p (h e) -> p h e", h=8))
            for ck in range(4 * n, 4 * n + 4):
                for g2 in range(4):
                    ps = psA.tile([128, 1024], F32, tag="pscore")
                    for j in range(2):
                        h = 2 * g2 + j
                        m, hh = h // 4, h % 4
                        nc.tensor.matmul(ps[:, j * 512:(j + 1) * 512],
                                         kpT_n[n][32 * hh:32 * hh + 32, m,
                                                  (ck % 4) * 128:(ck % 4) * 128 + 128],
                                         qpT[32 * hh:32 * hh + 32, m, :],
                                         start=True, stop=True, tile_position=(32 * hh, 0))
                    eb = ebp.tile([128, 1024], BF16, tag="eb")
                    nc.scalar.activation(eb[:], ps[:], AF.Exp)
                    first, last = (ck == 0), (ck == NKC - 1)
                    for j in range(2):
                        h = 2 * g2 + j
                        nc.tensor.matmul(oasc[g2][64 * j:64 * j + 33, :],
                                         vp_t[ck][:, h],
                                         eb[:, j * 512:(j + 1) * 512],
                                         start=first, stop=last,
                                         tile_position=(0, 64 * j),
                                         skip_group_check=True)

        # ---------------- value proj (after flash, psA staging) -----------
        # runs on the PE/ACT tail of flash while the finalize + post-attn
        # chain occupies the DVE; the table build (emitted later) waits on
        # it, well before the gathers need the tables.
        val = pinV.tile([128, 43, D], BF16, tag="val")
        nc.gpsimd.memset(val[:, 42], 0.0)
        for vck in range(42):
            pv2 = psA.tile([128, 512], F32, tag="pscore")
            for k in range(2):
                nc.tensor.matmul(pv2[:, 0:256], t_in["imgT_bf"][:, k, vck * 128:(vck + 1) * 128],
                                 t_in["valwT"][:, k], start=(k == 0), stop=(k == 1))
            nc.vector.tensor_copy(val[:, vck], pv2[:, 0:256])

        # Border strips: quad rows y'=0 / x'=0 still carry valid dy=1 / dx=1
        # corners (samples hanging off the top/left edge).
        for l, (hl, wl) in enumerate(SHAPES):
            c0, c1 = LVL_CHUNKS[l]
            w2 = W2S[l]
            ngrp = 128 // wl
            vh = val[:, c0:c1].rearrange("p c (h e) -> p c h e", h=8)
            # all 8 heads per DMA (quad-major rows keep head dim step=128)
            # top row y'=0: blocks (dy=1, dx): pixel (0, x'-1+dx)
            for dx in range(2):
                src = vh[0:wl, 0, :]
                base = (1 - dx) * 8 * 128 + (2 + dx) * 32
                dst = bass.AP(qt_l[l], base,
                              [[8 * 128, wl], [128, 8], [1, 32]])
                nc.sync.dma_start(dst, src)
            # left col x'=0: blocks (dy, dx=1): pixel (y'-1+dy, 0)
            for dy in range(2):
                for g in range(ngrp):
                    src = vh[g * wl:g * wl + 1, :, :, :]
                    base = ((1 - dy) + g) * w2 * 8 * 128 + (2 * dy + 1) * 32
                    dst = bass.AP(
                        qt_l[l], base,
                        [[ngrp * w2 * 8 * 128, c1 - c0],
                         [128, 8],
                         [1, 32]])
                    nc.sync.dma_start(dst, src)


        # finalize: broadcast sumexp rows to the 32-row head bands, divide
        srow = fls.tile([1, 8, LQC], BF16, tag="srow")
        for g2 in range(4):
            nc.vector.tensor_copy(srow[:, 2 * g2], oasc[g2][32:33, :])
            nc.vector.tensor_copy(srow[:, 2 * g2 + 1], oasc[g2][96:97, :])
        psA_cm.__exit__(None, None, None)
        psbc_cm = tc.tile_pool(name="psbc", bufs=1, space="PSUM")
        psbc = psbc_cm.__enter__()
        pbt0 = psbc.tile([128, LQC], F32, tag="psbc0")
        pbt1 = psbc.tile([128, LQC], F32, tag="psbc1")
        pbt = [pbt0, pbt1]
        ones1x32 = cst.tile([1, 32], BF16, tag="ones1x32")
        nc.gpsimd.memset(ones1x32[:], 1.0)
        for h in range(8):
            m, a = h // 4, h % 4
            nc.tensor.matmul(pbt[m][32 * a:32 * a + 32, :], ones1x32[:],
                             srow[:, h], start=True, stop=True,
                             tile_position=(0, 32 * a), skip_group_check=True)
        rsb = fls.tile([128, 2, LQC], F32, tag="rsb")
        ocat = res.tile([128, 2, LQC], BF16, tag="ocat")
        for m in range(2):
            nc.vector.reciprocal(rsb[:, m], pbt[m][:])
        for h in range(8):
            g2, j = h // 2, h % 2
            m, a = h // 4, h % 4
            nc.vector.tensor_mul(ocat[32 * a:32 * a + 32, m],
                                 oasc[g2][64 * j:64 * j + 32, :],
                                 rsb[32 * a:32 * a + 32, m])

        psbc_cm.__exit__(None, None, None)
        psacc_cm.__exit__(None, None, None)
        fls_cm.__exit__(None, None, None)
        post = ctx.enter_context(tc.tile_pool(name="post", bufs=1))
        post2 = ctx.enter_context(tc.tile_pool(name="post2", bufs=3))
        psB = ctx.enter_context(tc.tile_pool(name="psB", bufs=2, space="PSUM"))
        psD = ctx.enter_context(tc.tile_pool(name="psD", bufs=1, space="PSUM"))
        psS = ctx.enter_context(tc.tile_pool(name="psS", bufs=2, space="PSUM"))
        # level-2 table build first: tiny, and its gathers run first
        emit_build(2, psS)

        onesf = cst.tile([128, 1], F32, tag="onesf")
        nc.gpsimd.memset(onesf[:], 1.0)
        one1 = cst.tile([1, 1], F32, tag="one1")
        nc.gpsimd.memset(one1[:], 1.0)
        ones1x128 = cst.tile([1, 128], F32, tag="ones1x128")
        nc.gpsimd.memset(ones1x128[:], 1.0)

        def lnorm(pre, dst_f32, dst_bf, which, W=LQC):
            pm = psB.tile([128, W], F32, tag="ps512")
            for k in range(2):
                nc.tensor.matmul(pm[0:1, :], onesf[:], pre[:, k], start=(k == 0), stop=(k == 1))
            pm2 = psB.tile([128, W], F32, tag="ps512")
            for k in range(2):
                sq = post.tile([128, W], F32, tag="sq")
                nc.vector.tensor_mul(sq[:], pre[:, k], pre[:, k])
                nc.tensor.matmul(pm2[0:1, :], onesf[:], sq[:], start=(k == 0), stop=(k == 1))
            mean = post.tile([1, W], F32, tag="mean")
            nc.scalar.mul(mean[:], pm[0:1, :], 1.0 / D)
            var = post.tile([1, W], F32, tag="var")
            nc.scalar.mul(var[:], pm2[0:1, :], 1.0 / D)
            msq = post.tile([1, W], F32, tag="lv")
            nc.vector.tensor_mul(msq[:], mean[:], mean[:])
            nc.vector.tensor_sub(var[:], var[:], msq[:])
            nc.vector.tensor_scalar(var[:], var[:], EPS, None, ALU.add)
            lv = post.tile([1, W], F32, tag="lv")
            nc.scalar.activation(lv[:], var[:], AF.Ln)
            rstd = post.tile([1, W], F32, tag="rstd")
            nc.scalar.activation(rstd[:], lv[:], AF.Exp, scale=-0.5)
            m2 = post.tile([1, W], F32, tag="m2")
            nc.vector.tensor_mul(m2[:], mean[:], rstd[:])
            pb = psD.tile([128, 2 * W], F32, tag="pbc")
            nc.tensor.matmul(pb[:, 0:W], ones1x128[:], rstd[:], start=True, stop=True)
            nc.tensor.matmul(pb[:, W:2 * W], ones1x128[:], m2[:], start=True, stop=True)
            bca = post.tile([128, 2 * W], F32, tag="bca")
            nc.vector.tensor_copy(bca[:], pb[:])
            for k in range(2):
                tn = post.tile([128, W], F32, tag="tn")
                nc.vector.tensor_mul(tn[:], pre[:, k], bca[:, 0:W])
                nc.vector.tensor_sub(tn[:], tn[:], bca[:, W:2 * W])
                nc.vector.tensor_scalar(
                    dst_f32[:, k], tn[:],
                    t_in["lnp"][:, k, 2 * which:2 * which + 1],
                    t_in["lnp"][:, k, 2 * which + 1:2 * which + 2],
                    ALU.mult, ALU.add)
                if dst_bf is not None:
                    nc.vector.tensor_copy(dst_bf[:, k], dst_f32[:, k])

        x1 = post.tile([128, 2, LQC], F32, tag="x1")
        pre1 = post.tile([128, 2, LQC], F32, tag="pre")
        for m in range(2):
            po = psB.tile([128, LQC], F32, tag="ps512")
            for k in range(2):
                nc.tensor.matmul(po[:], t_in["woT"][:, k, m * 128:(m + 1) * 128],
                                 ocat[:, k], start=(k == 0), stop=(k == 1))
            nc.vector.tensor_add(pre1[:, m], t_in["bqcT"][:, m], po[:])
        lnorm(pre1, x1, None, 0)

        # ---------------- deformable ----------------
        q2 = post.tile([128, 2, LQC], BF16, tag="q2")
        for k in range(2):
            nc.vector.tensor_add(q2[:, k], x1[:, k], t_in["poscT"][:, k])

        offq = post.tile([128, NQC, 192], F32, tag="offq")
        awq = post.tile([128, NQC, 96], F32, tag="awq")
        for qc in range(NQC):
            pof = psB.tile([128, 512], F32, tag="ps512")
            for k in range(2):
                nc.tensor.matmul(pof[:, 0:192], q2[:, k, qc * 128:(qc + 1) * 128],
                                 t_in["offwT"][:, k], start=(k == 0), stop=False)
            nc.tensor.matmul(pof[:, 0:192], ones1x128[:], t_in["offb"][:],
                             start=False, stop=True)
            nc.vector.tensor_copy(offq[:, qc], pof[:, 0:192])
            paw = psB.tile([128, 512], F32, tag="ps512")
            for k in range(2):
                nc.tensor.matmul(paw[:, 0:96], q2[:, k, qc * 128:(qc + 1) * 128],
                                 t_in["attnwT"][:, k], start=(k == 0), stop=(k == 1))
            eaw = post.tile([128, 96], F32, tag="eaw")
            nc.scalar.activation(eaw[:], paw[:, 0:96], AF.Exp)
            sm = post.tile([128, 8], F32, tag="sm")
            nc.vector.tensor_reduce(sm[:], eaw[:].rearrange("p (h s) -> p h s", h=8),
                                    mybir.AxisListType.X, ALU.add)
            rsm = post.tile([128, 8], F32, tag="rsm")
            nc.vector.reciprocal(rsm[:], sm[:])
            nc.vector.tensor_mul(awq[:, qc].rearrange("p (h s) -> p h s", h=8),
                                 eaw[:].rearrange("p (h s) -> p h s", h=8),
                                 rsm[:].unsqueeze(2).broadcast_to([128, 8, 12]))

        ocaq = post.tile([128, NQC, D], F32, tag="ocaq")
        for qc in range(NQC):
            nc.gpsimd.memset(ocaq[:, qc], 0.0)

        qnames = ["qPoolDynamic", "qPoolDynamic1", "qPoolDynamic2",
                  "qPoolDynamic3"]
        # pass 1: compute gather indices + corner weights, batched over all
        # NQC query chunks per level (4x fewer DVE instructions).
        idx_t = {}
        cw_t = {}
        NJ = NQC * 32

        def emit_pass1(l):
            hl, wl = SHAPES[l]
            w2 = W2S[l]
            if True:
                off6 = offq[:].rearrange("p q (h l k two) -> p q h l k two",
                                         h=8, l=3, k=4)
                xo = off6[:, :, :, l, :, 0]
                yo = off6[:, :, :, l, :, 1]
                refx = post.tile([128, NQC, 1], F32, tag="refx")
                nc.vector.tensor_scalar(refx[:], t_in["refq"][:, :, 2 * l:2 * l + 1],
                                        float(wl), -0.5, ALU.mult, ALU.add)
                refy = post.tile([128, NQC, 1], F32, tag="refy")
                nc.vector.tensor_scalar(refy[:], t_in["refq"][:, :, 2 * l + 1:2 * l + 2],
                                        float(hl), -0.5, ALU.mult, ALU.add)
                xs = post.tile([128, NQC, 32], F32, tag="xs")
                nc.vector.tensor_tensor(
                    xs[:].rearrange("p q (h c) -> p q h c", h=8), xo,
                    refx[:].unsqueeze(3).broadcast_to([128, NQC, 8, 4]), ALU.add)
                ys = post.tile([128, NQC, 32], F32, tag="ys")
                nc.vector.tensor_tensor(
                    ys[:].rearrange("p q (h c) -> p q h c", h=8), yo,
                    refy[:].unsqueeze(3).broadcast_to([128, NQC, 8, 4]), ALU.add)

                def floorpair(src, tag):
                    # robust floor: t = int(src+16); tf = float(t) - 16;
                    # if tf > src: tf -= 1   (works for trunc or round)
                    ti = post.tile([128, NQC, 32], I32, tag=tag + "i")
                    tsh = post.tile([128, NQC, 32], F32, tag=tag + "sh")
                    nc.vector.tensor_scalar(tsh[:], src[:], 16.0, None, ALU.add)
                    nc.vector.tensor_copy(ti[:], tsh[:])
                    tf = post.tile([128, NQC, 32], F32, tag=tag + "f")
                    nc.vector.tensor_copy(tf[:], ti[:])
                    nc.vector.tensor_scalar(tf[:], tf[:], -16.0, None, ALU.add)
                    gt = post.tile([128, NQC, 32], F32, tag=tag + "g")
                    nc.vector.tensor_tensor(gt[:], tf[:], src[:], ALU.is_gt)
                    nc.vector.tensor_sub(tf[:], tf[:], gt[:])
                    w = post.tile([128, NQC, 32], F32, tag=tag + "w")
                    nc.vector.tensor_sub(w[:], src[:], tf[:])
                    return tf, w

                x0f, wx = floorpair(xs, "fx")
                y0f, wy = floorpair(ys, "fy")

                def vmask(base_f, hi, tag):
                    v0 = post.tile([128, NQC, 32], F32, tag=tag + "v0")
                    nc.vector.tensor_scalar(v0[:], base_f[:], 0.0, None, ALU.is_ge)
                    t = post.tile([128, NQC, 32], F32, tag=tag + "t")
                    nc.vector.tensor_scalar(t[:], base_f[:], float(hi - 1), None, ALU.is_le)
                    nc.vector.tensor_mul(v0[:], v0[:], t[:])
                    v1 = post.tile([128, NQC, 32], F32, tag=tag + "v1")
                    nc.vector.tensor_scalar(v1[:], base_f[:], -1.0, None, ALU.is_ge)
                    nc.vector.tensor_scalar(t[:], base_f[:], float(hi - 2), None, ALU.is_le)
                    nc.vector.tensor_mul(v1[:], v1[:], t[:])
                    return v0, v1

                vx0, vx1 = vmask(x0f, wl, "vx")
                vy0, vy1 = vmask(y0f, hl, "vy")

                awt = post.tile([128, NQC, 32], F32, tag="awt")
                nc.vector.tensor_copy(
                    awt[:].rearrange("p q (h c) -> p q h c", h=8),
                    awq[:].rearrange("p q (h s) -> p q h s", h=8)[:, :, :, l * 4:(l + 1) * 4])

                wx0 = post.tile([128, NQC, 32], F32, tag="wx0")
                nc.vector.tensor_scalar(wx0[:], wx[:], -1.0, 1.0, ALU.mult, ALU.add)
                nc.vector.tensor_mul(wx0[:], wx0[:], vx0[:])
                wx1 = post.tile([128, NQC, 32], F32, tag="wx1")
                nc.vector.tensor_mul(wx1[:], wx[:], vx1[:])
                wy0 = post.tile([128, NQC, 32], F32, tag="wy0")
                nc.vector.tensor_scalar(wy0[:], wy[:], -1.0, 1.0, ALU.mult, ALU.add)
                nc.vector.tensor_mul(wy0[:], wy0[:], vy0[:])
                nc.vector.tensor_mul(wy0[:], wy0[:], awt[:])
                wy1 = post.tile([128, NQC, 32], F32, tag="wy1")
                nc.vector.tensor_mul(wy1[:], wy[:], vy1[:])
                nc.vector.tensor_mul(wy1[:], wy1[:], awt[:])

                cw = post.tile([128, NQC, 32, 4], F32, tag=f"cw{l}")
                cw_t[l] = cw
                nc.vector.tensor_mul(cw[:, :, :, 0], wy0[:], wx0[:])
                nc.vector.tensor_mul(cw[:, :, :, 1], wy0[:], wx1[:])
                nc.vector.tensor_mul(cw[:, :, :, 2], wy1[:], wx0[:])
                nc.vector.tensor_mul(cw[:, :, :, 3], wy1[:], wx1[:])

                # float index (quad-major rows):
                #   (clip(y0+1,0,hl)*w2 + clip(x0+1,0,wl))*8 + h  (cb = h)
                xcf = post.tile([128, NQC, 32], F32, tag="xcf")
                nc.vector.tensor_scalar(xcf[:], x0f[:], 1.0, 0.0, ALU.add, ALU.max)
                nc.vector.tensor_scalar(xcf[:], xcf[:], float(wl), 8.0, ALU.min, ALU.mult)
                ycf = post.tile([128, NQC, 32], F32, tag="ycf")
                nc.vector.tensor_scalar(ycf[:], y0f[:], 1.0, 0.0, ALU.add, ALU.max)
                nc.vector.tensor_scalar(ycf[:], ycf[:], float(hl), None, ALU.min)
                idxf = post.tile([128, NQC, 32], F32, tag="idxf")
                nc.vector.tensor_scalar(idxf[:], ycf[:], float(w2 * 8), None, ALU.mult)
                nc.vector.tensor_add(idxf[:], idxf[:], xcf[:])
                nc.vector.tensor_add(
                    idxf[:], idxf[:],
                    t_in[f"cb{l}"][:].unsqueeze(1).broadcast_to([128, NQC, 32]))
                idx = post.tile([128, NQC, 32], I32, tag=f"idx{l}")
                idx_t[l] = idx
                nc.vector.tensor_copy(idx[:], idxf[:])

        # pass 2: per level (2 -> 1 -> 0): table build (PE shifts + scalar
        # copies, so the DVE stream stays free for interpolation), then the
        # level's gathers + interpolation. Emission order: a gather waits on
        # every earlier-emitted qt write (per-tensor DRAM deps), so each
        # level's build is emitted right before its own gathers. After a qc's
        # last chunk (level 0), its transpose + ca-out projection run under
        # the remaining gathers.
        ocab = post.tile([128, NQC, D], BF16, tag="ocab")
        ocaT = post.tile([128, 2, LQC], BF16, tag="ocaT")
        pre2 = post.tile([128, 2, LQC], F32, tag="pre")
        gq_t = {}

        _gq_ctr = [0]

        def emit_gather(l, qc):
            # one batched indirect DMA: 4096 offsets (32 rows x 128
            # partitions) in a single instruction -- the SWDGE per-call
            # fixed cost is paid once instead of 32 times.
            gq = post2.tile([128, 32, 128], BF16, tag="gq")
            gq_t[(l, qc)] = gq
            if BATCH_GATHER:
                gi_inst = nc.gpsimd.indirect_dma_start(
                    gq[:], None, qta_l[l],
                    IndirectOffsetOnAxis(ap=idx_t[l][:, qc], axis=0))
                gi_inst.ins.queue = qnames[_gq_ctr[0] % 4]
                _gq_ctr[0] += 1
            else:
                idx = idx_t[l]
                for j in range(32):
                    gi_inst = nc.gpsimd.indirect_dma_start(
                        gq[:, j, :], None, qta_l[l],
                        IndirectOffsetOnAxis(ap=idx[:, qc, j:j + 1], axis=0))
                    gi_inst.ins.queue = qnames[j % 4]

        def emit_interp(l, qc):
            if True:
                cw = cw_t[l][:, qc]
                gq = gq_t[(l, qc)]
                tmp = post.tile([128, 32, 4, 32], BF16, tag="tmpc")
                nc.vector.tensor_mul(
                    tmp[:], gq[:].rearrange("p j (s c) -> p j s c", s=4),
                    cw.unsqueeze(3).broadcast_to([128, 32, 4, 32]))
                red = post.tile([128, 8, 32], F32, tag="red")
                nc.vector.tensor_reduce(
                    red[:],
                    tmp[:].rearrange("p (h pp) s c -> p h c pp s", h=8),
                    mybir.AxisListType.XY, ALU.add)
                nc.vector.tensor_add(
                    ocaq[:, qc].rearrange("p (h c) -> p h c", h=8),
                    ocaq[:, qc].rearrange("p (h c) -> p h c", h=8), red[:])
                if l == 0:
                    # qc complete: transpose + ca-out proj columns now
                    nc.vector.tensor_copy(ocab[:, qc], ocaq[:, qc])
                    for dc in range(2):
                        pt = psD.tile([128, 128], BF16, tag="ptc")
                        nc.tensor.transpose(pt[:], ocab[:, qc, dc * 128:(dc + 1) * 128],
                                            t_in["idt"][:])
                        nc.vector.tensor_copy(ocaT[:, dc, qc * 128:(qc + 1) * 128],
                                              pt[:])
                    qs = slice(qc * 128, (qc + 1) * 128)
                    for m in range(2):
                        pc = psB.tile([128, 512], F32, tag="ps512")
                        for k in range(2):
                            nc.tensor.matmul(pc[:, 0:128],
                                             t_in["cawT"][:, k, m * 128:(m + 1) * 128],
                                             ocaT[:, k, qs],
                                             start=(k == 0), stop=(k == 1))
                        nc.vector.tensor_add(pre2[:, m, qs], x1[:, m, qs], pc[:, 0:128])
                    emit_tail(qc)

        # per-chunk tail: lnorm2 + FFN + lnorm3 + store for one 128-query
        # chunk, emitted as soon as its last interp + ca-out are in -- the
        # final chunk's tail is all that remains after the gather stream.
        x2 = post.tile([128, 2, LQC], F32, tag="x2")
        x2b = post.tile([128, 2, LQC], BF16, tag="x2b")
        h1 = post.tile([128, 4, LQC], BF16, tag="h1")
        pre3 = post.tile([128, 2, LQC], F32, tag="pre3")
        x3 = post.tile([128, 2, LQC], F32, tag="x3")

        def emit_tail(qc):
            qs = slice(qc * 128, (qc + 1) * 128)
            lnorm(pre2[:, :, qs], x2[:, :, qs], x2b[:, :, qs], 1, W=128)
            for m in range(4):
                ph = psB.tile([128, 128], F32, tag="ps512")
                for k in range(2):
                    nc.tensor.matmul(ph[:], t_in["w1T"][:, k, m * 128:(m + 1) * 128],
                                     x2b[:, k, qs], start=(k == 0), stop=(k == 1))
                nc.vector.tensor_scalar(h1[:, m, qs], ph[:], 0.0, None, ALU.max)
            for m in range(2):
                pf = psB.tile([128, 128], F32, tag="ps512")
                for k in range(4):
                    nc.tensor.matmul(pf[:], t_in["w2T"][:, k, m * 128:(m + 1) * 128],
                                     h1[:, k, qs], start=(k == 0), stop=(k == 3))
                nc.vector.tensor_add(pre3[:, m, qs], x2[:, m, qs], pf[:])
            lnorm(pre3[:, :, qs], x3[:, :, qs], None, 2, W=128)
            for k in range(2):
                nc.sync.dma_start(outT.rearrange("(c p) n -> p c n", p=128)[:, k, qs],
                                  x3[:, k, qs])

        # schedule: all pass-1 index math up front (before any gather, so a
        # gather's conservative wait on earlier-emitted vector ops is already
        # satisfied), then table builds for levels 1/0 (level 2 was built
        # early, during flash), then the gather stream, CHUNK-major so each
        # 128-query chunk finishes (interp + ca-out + lnorm2/FFN/store) while
        # later chunks still gather; interps trail by 1 for gq buffering.
        CHUNKS = [(l_, qc_) for qc_ in range(NQC) for l_ in (2, 1, 0)]
        emit_pass1(2)
        emit_pass1(1)
        emit_pass1(0)
        emit_build(1, psS)
        emit_build(0, psS)
        for i_, (l_, qc_) in enumerate(CHUNKS):
            emit_gather(l_, qc_)
            if i_ >= 1:
                emit_interp(*CHUNKS[i_ - 1])
        emit_interp(*CHUNKS[-1])

    if split_waits:
        _split_multi_waits(nc)
    return nc


_NC_CACHE = {}
TRACE = False


def kernel(**inputs):
    bq = np.asarray(inputs["bev_queries"])[0]
    img = np.asarray(inputs["img_feats_flat"])[0]
    ref = np.asarray(inputs["ref_points"])[0]
    pos = np.asarray(inputs["bev_pos"])[0]
    miw = np.asarray(inputs["mha_in_w"]); mow = np.asarray(inputs["mha_out_w"])
    offw = np.asarray(inputs["off_w"]); offb = np.asarray(inputs["off_b"])
    attw = np.asarray(inputs["attn_w"])
    valw = np.asarray(inputs["val_w"]); caw = np.asarray(inputs["ca_out_w"])
    w1 = np.asarray(inputs["ffn_w1"]); w2 = np.asarray(inputs["ffn_w2"])

    bf = ml_dtypes.bfloat16
    common = {
        "bqT_bf": np.ascontiguousarray(bq.T).astype(bf),
        "posT_bf": np.ascontiguousarray(pos.T).astype(bf),
        "imgT_bf": np.ascontiguousarray(img.T).astype(bf),
        "wqT": np.ascontiguousarray((miw[0:256] / np.sqrt(32.0)).T).astype(bf),
        "wkT": np.ascontiguousarray(miw[256:512].T).astype(bf),
        "wvT": np.ascontiguousarray(miw[512:768].T).astype(bf),
        "woT": np.ascontiguousarray(mow.T).astype(bf),
        "offwT": np.ascontiguousarray(offw.T).astype(bf),
        "offb": offb.reshape(1, 192).astype(np.float32),
        "attnwT": np.ascontiguousarray(attw.T).astype(bf),
        "valwT": np.ascontiguousarray(valw.T).astype(bf),
        "cawT": np.ascontiguousarray(caw.T).astype(bf),
        "w1T": np.ascontiguousarray(w1.T).astype(bf),
        "w2T": np.ascontiguousarray(w2.T).astype(bf),
        "idt": np.eye(128).astype(bf),
    }
    pshift = np.zeros((128, 14, 128), np.float32)
    for si, sh in enumerate([1, 16, 17, 32, 33, 64, 65]):
        for m_ in range(128 - sh):
            pshift[m_ + sh, 2 * si, m_] = 1.0      # main: out[m] = in[m+sh]
        for m_ in range(128 - sh, 128):
            pshift[m_ + sh - 128, 2 * si + 1, m_] = 1.0  # wrap
    common["pshift"] = pshift.reshape(128, 14 * 128).astype(bf)
    lnp = np.zeros((128, 2, 6), np.float32)
    prs = [(inputs["norm1_g"], inputs["norm1_b"]),
           (inputs["norm2_g"], inputs["norm2_b"]),
           (inputs["ffn_g"], inputs["ffn_bb"])]
    for k in range(2):
        sl = slice(k * 128, (k + 1) * 128)
        for i, (g, b) in enumerate(prs):
            lnp[:, k, 2 * i] = np.asarray(g)[sl]
            lnp[:, k, 2 * i + 1] = np.asarray(b)[sl]
    common["lnp"] = lnp
    for l in range(3):
        cb = np.zeros((128, 32), np.float32)
        for h in range(8):
            cb[:, h * 4:(h + 1) * 4] = h  # quad-major rows: idx = quad*8 + h
        common[f"cb{l}"] = cb

    if "nc" not in _NC_CACHE:
        _NC_CACHE["nc"] = build_kernel()
    nc = _NC_CACHE["nc"]

    in_maps = []
    for c in range(8):
        sl = slice(c * LQC, (c + 1) * LQC)
        m = dict(common)
        m["bqcT"] = np.ascontiguousarray(bq[sl].T).astype(np.float32)
        m["poscT"] = np.ascontiguousarray(pos[sl].T).astype(np.float32)
        refc = ref[sl].reshape(NQC, 128, 6)
        m["refq"] = np.ascontiguousarray(refc.transpose(1, 0, 2)).astype(np.float32)
        in_maps.append(m)

    res = run_bass_kernel_spmd(nc, in_maps, list(range(8)), trace=TRACE)
    _NC_CACHE["last_res"] = res
    out = np.zeros((1, LQ, D), np.float32)
    for c in range(8):
        out[0, c * LQC:(c + 1) * LQC, :] = res.results[c]["outT"].T
    return out



# revision 41
# speedup vs baseline: 1.0091x; 1.0091x over previous
"""BEVFormer encoder layer on 8 Trainium2 NeuronCores (Bass/Tile, SPMD).

Sharding: data-parallel over the 4096 BEV queries (512 per core); K/V and the
deformable value table are computed redundantly per core, so no collectives.
Large matmuls run in bf16 (fp32 is 4x slower on the PE); the residual stream,
layernorms, softmax statistics and sampling arithmetic stay fp32. Deformable
sampling uses a DRAM "quad" table (each row = a 2x2 pixel neighborhood of one
head, padded ring; quad-major rows so all 8 heads of a quad are contiguous)
gathered with native indirect DMA (32 calls per level x query-chunk, 128
offsets each, spread over 4 SWDGE queues). Flash self-attention accumulates
attn@V and the softmax denominator directly in PSUM across key chunks.
"""
import numpy as np
import ml_dtypes
from contextlib import ExitStack

import concourse.bass as bass
import concourse.tile as tile
from concourse import mybir
from concourse.bass import IndirectOffsetOnAxis

from concourse.bass_utils import run_bass_kernel_spmd

# ---------------------------------------------------------------------------
# Inlined toolchain workarounds (kernel.py must be self-contained).
# (1) NTFF profile hook for trace=True under axon (stub antenv lacks it).
# (2) walrus in this image allows only ONE sem-wait per instruction; Tile
#     emits multi-wait instructions, so hoist extras onto same-engine nops.
# ---------------------------------------------------------------------------
import contextlib as _ctxlib
import ctypes as _ctypes
import sys as _sys
import types as _types

def _install_ntff_hook():
    so_path = "/opt/axon/libaxon_pjrt.so"
    try:
        lib = _ctypes.CDLL(so_path)
    except OSError:
        lib = None
    if lib is None or not hasattr(lib, "axon_start_nrt_profile"):
        hook = None
    else:
        lib.axon_start_nrt_profile.argtypes = [_ctypes.POINTER(_ctypes.c_int64), _ctypes.c_size_t]
        lib.axon_start_nrt_profile.restype = _ctypes.c_int64
        lib.axon_stop_nrt_profile.argtypes = [_ctypes.c_char_p]
        lib.axon_stop_nrt_profile.restype = _ctypes.c_int64

        @_ctxlib.contextmanager
        def hook(output_dir, device_ids):
            import jax
            jax.devices()
            if device_ids:
                ids = (_ctypes.c_int64 * len(device_ids))(*device_ids)
                rc = lib.axon_start_nrt_profile(ids, len(device_ids))
            else:
                rc = lib.axon_start_nrt_profile(None, 0)
            if rc != 0:
                raise RuntimeError(f"axon_start_nrt_profile rc={rc}")
            try:
                yield
            finally:
                n = lib.axon_stop_nrt_profile(str(output_dir).encode())
                if n < 0:
                    raise RuntimeError(f"axon_stop_nrt_profile rc={n}")

    mod = _types.ModuleType("antenv.axon_hooks")
    mod.get_axon_ntff_profile_hook = lambda: hook
    mod.set_axon_ntff_profile_hook = lambda h: None
    _sys.modules["antenv.axon_hooks"] = mod

_install_ntff_hook()

from concourse.tile import ScopedClock as _ScopedClock

def _patched_drain_and_barrier(self, tick_clock, wait_clock):
    nc_ = self.nc
    drain_inst = nc_.sync.drain()
    inst = drain_inst.ins
    wait_clock.add_sem_waits(inst, _ScopedClock({None: tick_clock.global_clock}))
    si = inst.sync_info
    waits = list(si.on_wait or []) if si else []
    if len(waits) > 1:
        si.on_wait = waits[:1]
        inst.sync_info = si
        for w in waits[1:]:
            nop = nc_.sync.nop()
            ni = nop.ins
            ni.sync_info = mybir.SyncInfo(on_wait=[w], on_update=[])
    nc_.all_engine_barrier()
    assert self.sems is not None
    popped = nc_._tile_sem_poison_stack.pop()
    assert popped is self._sem_poison
    nc_.clear_and_free_semaphores(list(self.sems.allocated().values()))
    nc_.all_engine_barrier()

tile.TileContext._drain_and_barrier = _patched_drain_and_barrier

def _split_multi_waits(nc_, max_waits=1):
    n_split = 0
    for f in nc_.m.functions:
        for bb in f.blocks:
            out = []
            changed = False
            for inst in bb.instructions:
                si = inst.sync_info
                waits = list(si.on_wait) if si and si.on_wait else []
                if len(waits) > max_waits:
                    reg_waits = [w for w in waits if w.wait_reg is not None]
                    imm_waits = [w for w in waits if w.wait_reg is None]
                    keep_n = max(0, max_waits - len(reg_waits))
                    nh = len(imm_waits) - keep_n if keep_n < len(imm_waits) else 0
                    hoist = imm_waits[:nh]
                    if hoist:
                        changed = True
                        for w in hoist:
                            n_split += 1
                            nop = mybir.InstNoOp(name=f"waitsplit-{n_split}")
                            nop.engine = inst.engine
                            nop.sync_info = mybir.SyncInfo(on_wait=[w], on_update=[])
                            out.append(nop)
                        si.on_wait = reg_waits + imm_waits[nh:]
                        inst.sync_info = si
                out.append(inst)
            if changed:
                bb.instructions = out
    return n_split


F32 = mybir.dt.float32
BF16 = mybir.dt.bfloat16
I32 = mybir.dt.int32
AF = mybir.ActivationFunctionType
ALU = mybir.AluOpType

D = 256
H = 8
LQ = 4096
LQC = 512          # queries per core
NQC = LQC // 128   # q-chunks per core
LV = 5376
DFFN = 512
SHAPES = [(64, 64), (32, 32), (16, 16)]
LVL_CHUNKS = [(0, 32), (32, 40), (40, 42)]
W2S = [w + 2 for _, w in SHAPES]
R_L = [(h + 2) * (w + 2) for h, w in SHAPES]
QB_L = [0, 8 * R_L[0], 8 * (R_L[0] + R_L[1])]
QT_ROWS = 8 * sum(R_L)
NKC = 32
EPS = 1e-5
BATCH_GATHER = False


def build_kernel(split_waits=True):
    nc = bass.Bass("TRN2", target_bir_lowering=False, debug=False, num_devices=8,
                   num_swdge_queues=4)

    din = {}
    def dI(name, shape, dt=F32):
        din[name] = nc.dram_tensor(name, shape, dt, kind="ExternalInput").ap()

    dI("bqT_bf", [D, LQ], BF16)
    dI("posT_bf", [D, LQ], BF16)
    dI("imgT_bf", [D, LV], BF16)
    dI("bqcT", [D, LQC], F32)
    dI("poscT", [D, LQC], F32)
    dI("refq", [128, NQC, 6], F32)
    dI("wqT", [D, D], BF16)
    dI("wkT", [D, D], BF16)
    dI("wvT", [D, D], BF16)
    dI("woT", [D, D], BF16)
    dI("offwT", [D, 192], BF16)
    dI("offb", [1, 192], F32)
    dI("attnwT", [D, 96], BF16)
    dI("valwT", [D, D], BF16)
    dI("cawT", [D, D], BF16)
    dI("w1T", [D, DFFN], BF16)
    dI("w2T", [DFFN, D], BF16)
    dI("lnp", [128, 2, 6], F32)
    dI("pshift", [128, 14 * 128], BF16)
    dI("cb0", [128, 32], F32)
    dI("cb1", [128, 32], F32)
    dI("cb2", [128, 32], F32)
    dI("idt", [128, 128], BF16)

    outT = nc.dram_tensor("outT", [D, LQC], F32, kind="ExternalOutput").ap()
    # one quad table per level: gathers of level l then only depend on that
    # level's writes (Tile DRAM deps are per-tensor)
    qt_l = [nc.dram_tensor(f"qt{l}", [8 * R_L[l], 128], BF16) for l in range(3)]
    qta_l = [t.ap() for t in qt_l]

    BIG = ("bqT_bf", "posT_bf", "imgT_bf")
    with tile.TileContext(nc) as tc, ExitStack() as ctx:
        cst = ctx.enter_context(tc.tile_pool(name="cst", bufs=1))
        res = ctx.enter_context(tc.tile_pool(name="res", bufs=1))
        ebp = ctx.enter_context(tc.tile_pool(name="ebp", bufs=2))
        pinV = ctx.enter_context(tc.tile_pool(name="pinV", bufs=1))
        vq = ctx.enter_context(tc.tile_pool(name="vq", bufs=1))
        # flash-only tensors: freed after the flash finalize so the
        # deformable pools (gq double-buffer etc.) fit in SBUF.
        # opened BEFORE pin so pool releases stay LIFO (pin closes first).
        fls_cm = tc.tile_pool(name="fls", bufs=1)
        fls = fls_cm.__enter__()
        pin_cm = tc.tile_pool(name="pin", bufs=1)
        pin = pin_cm.__enter__()
        psP_cm = tc.tile_pool(name="psP", bufs=2, space="PSUM")
        psP = psP_cm.__enter__()

        # ---------------- load inputs ----------------
        t_in = {}
        for name, ap in din.items():
            shp = list(ap.shape)
            pool_ = pin if name in BIG else cst
            if shp[0] > 128:
                tl = pool_.tile([128, shp[0] // 128, shp[1]], ap.dtype, tag=name)
                nc.sync.dma_start(tl[:], ap.rearrange("(c p) n -> p c n", p=128))
            else:
                tl = pool_.tile(shp, ap.dtype, tag=name)
                nc.sync.dma_start(tl[:], ap[:])
            t_in[name] = tl

        # this core's q block (f32 add then bf16)
        qcT = fls.tile([128, 2, LQC], BF16, tag="qcT")
        for c in range(2):
            nc.vector.tensor_add(qcT[:, c], t_in["bqcT"][:, c], t_in["poscT"][:, c])

        # ---------------- kpT [256, 4096] bf16 ----------------
        # k-proj of (bq + pos) with the add folded into the PSUM accumulation
        kpT = fls.tile([128, 2, LQ], BF16, tag="kpT")
        for m in range(2):
            for n in range(8):
                pk = psP.tile([128, 512], F32, tag="ps512")
                for ki, src in enumerate(("bqT_bf", "posT_bf")):
                    for k in range(2):
                        nc.tensor.matmul(pk[:], t_in["wkT"][:, k, m * 128:(m + 1) * 128],
                                         t_in[src][:, k, n * 512:(n + 1) * 512],
                                         start=(ki == 0 and k == 0),
                                         stop=(ki == 1 and k == 1))
                nc.vector.tensor_copy(kpT[:, m, n * 512:(n + 1) * 512], pk[:])

        # ---------------- qpT [256, 512] bf16 ----------------
        qpT = fls.tile([128, 2, LQC], BF16, tag="qpT")
        for m in range(2):
            pq = psP.tile([128, 512], F32, tag="ps512")
            for k in range(2):
                nc.tensor.matmul(pq[:], t_in["wqT"][:, k, m * 128:(m + 1) * 128],
                                 qcT[:, k], start=(k == 0), stop=(k == 1))
            nc.vector.tensor_copy(qpT[:, m], pq[:])

        # ---------------- vp [4096, 8, 33] bf16 (col 32 = ones for sumexp) ----
        vp = fls.tile([128, NKC, 8, 33], BF16, tag="vp")
        nc.gpsimd.memset(vp[:, :, :, 32:33], 1.0)
        for ck in range(NKC):
            pv = psP.tile([128, 512], F32, tag="ps512")
            for k in range(2):
                nc.tensor.matmul(pv[:, 0:256], t_in["bqT_bf"][:, k, ck * 128:(ck + 1) * 128],
                                 t_in["wvT"][:, k], start=(k == 0), stop=(k == 1))
            nc.vector.tensor_copy(vp[:, ck, :, 0:32],
                                  pv[:, 0:256].rearrange("p (h e) -> p h e", h=8))

        # ---------------- value proj ----------------
        # chunk 42 is a zero pad so corner-shift reads past the last chunk
        # stay in-bounds (the shifted-in cells are weight-masked anyway).
        val = pinV.tile([128, 43, D], BF16, tag="val")
        nc.gpsimd.memset(val[:, 42], 0.0)
        for ck in range(42):
            pv2 = psP.tile([128, 512], F32, tag="ps512")
            for k in range(2):
                nc.tensor.matmul(pv2[:, 0:256], t_in["imgT_bf"][:, k, ck * 128:(ck + 1) * 128],
                                 t_in["valwT"][:, k], start=(k == 0), stop=(k == 1))
            # scalar-engine copy: the quad-table build copies (also scalar)
            # then depend on val in-order, avoiding Tile's conservative
            # emission-position cross-engine waits.
            nc.scalar.copy(val[:, ck], pv2[:, 0:256])

        # ---------------- quad table ----------------
        # Zero-fill (ring borders must read as 0.0); interior rows are
        # overwritten below.
        zt = pinV.tile([128, 512], BF16, tag="zt")
        nc.gpsimd.memset(zt[:], 0.0)
        for l in range(3):
            zflat = qta_l[l].rearrange("r c -> (r c)")
            total = 8 * R_L[l] * 128
            step = 128 * 512
            o = 0
            while o < total:
                n = min(step, total - o)
                if n % 128 == 0:
                    nc.sync.dma_start(zflat[o:o + n].rearrange("(p f) -> p f", p=128),
                                      zt[:, 0:n // 128])
                else:
                    nc.sync.dma_start(zflat[o:o + n].rearrange("(p f) -> p f", p=1),
                                      zt[0:1, 0:n])
                o += n

        # Quad-table interior: build per-slice corner-interleaved tiles
        # vqh[p, c, h, 4crn, 32] in SBUF, then write the table with
        # 256B-contiguous runs (4x fewer/4x bigger HBM descriptors than
        # scattering 64B corner blocks). The partition shifts (+1, +wl, +wl+1
        # pixels) run on the PE as permutation matmuls (pshift[:, 2i]/[:, 2i+1]
        # = main/wrap shifted identities for shift SHV[i]). Deferred until
        # after flash (level 2 first) so it stays off the critical chain:
        # gathers run level 2 -> 1 -> 0 while level 0's table finishes.
        SHV = [1, 16, 17, 32, 33, 64, 65]
        psh = t_in["pshift"][:].rearrange("p (s c) -> p s c", c=128)

        def emit_build(l, pspool):
            hl, wl = SHAPES[l]
            c0, c1 = LVL_CHUNKS[l]
            w2 = W2S[l]
            ngrp = 128 // wl
            nsl = (c1 - c0 + 7) // 8
            for s in range(nsl):
                sc0 = c0 + s * 8
                scn = min(8, c1 - sc0)
                vqh = vq.tile([128, 8, 8, 4, 32], BF16, tag="vqh")
                nc.scalar.copy(
                    vqh[:, 0:scn, :, 0],
                    val[:, sc0:sc0 + scn].rearrange("p c (h e) -> p c h e", h=8))
                for dy in range(2):
                    for dx in range(2):
                        sh = dy * wl + dx
                        if sh == 0:
                            continue
                        si = SHV.index(sh)
                        for brel in range(0, scn, 2):
                            cA = sc0 + brel
                            pp = pspool.tile([128, 512], F32, tag="pss")
                            rhs_m = val[:, cA:cA + 2].rearrange("p c e -> p (c e)")
                            rhs_w = val[:, cA + 1:cA + 3].rearrange("p c e -> p (c e)")
                            nc.tensor.matmul(pp[:], psh[:, 2 * si], rhs_m,
                                             start=True, stop=False)
                            nc.tensor.matmul(pp[:], psh[:, 2 * si + 1], rhs_w,
                                             start=False, stop=True)
                            nc.scalar.copy(
                                vqh[:, brel:brel + 2, :, 2 * dy + dx],
                                pp[:].rearrange("p (c h e) -> p c h e", c=2, h=8))
                for g in range(ngrp):
                    # quad-major table rows (row = quad*8 + h): all 8 heads
                    # are 1024 contiguous elems -> one 3-dim DMA per group
                    srcap = vqh[g * wl:(g + 1) * wl, 0:scn, :]
                    y0 = (sc0 - c0) * ngrp + g
                    base = ((y0 + 1) * w2 + 1) * 8 * 128
                    dst = bass.AP(
                        qt_l[l], base,
                        [[8 * 128, wl],
                         [ngrp * w2 * 8 * 128, scn],
                         [1, 8 * 128]])
                    nc.sync.dma_start(dst, srcap)

        # Border strips: quad rows y'=0 / x'=0 still carry valid dy=1 / dx=1
        # corners (samples hanging off the top/left edge).
        for l, (hl, wl) in enumerate(SHAPES):
            c0, c1 = LVL_CHUNKS[l]
            w2 = W2S[l]
            ngrp = 128 // wl
            vh = val[:, c0:c1].rearrange("p c (h e) -> p c h e", h=8)
            # all 8 heads per DMA (quad-major rows keep head dim step=128)
            # top row y'=0: blocks (dy=1, dx): pixel (0, x'-1+dx)
            for dx in range(2):
                src = vh[0:wl, 0, :]
                base = (1 - dx) * 8 * 128 + (2 + dx) * 32
                dst = bass.AP(qt_l[l], base,
                              [[8 * 128, wl], [128, 8], [1, 32]])
                nc.sync.dma_start(dst, src)
            # left col x'=0: blocks (dy, dx=1): pixel (y'-1+dy, 0)
            for dy in range(2):
                for g in range(ngrp):
                    src = vh[g * wl:g * wl + 1, :, :, :]
                    base = ((1 - dy) + g) * w2 * 8 * 128 + (2 * dy + 1) * 32
                    dst = bass.AP(
                        qt_l[l], base,
                        [[ngrp * w2 * 8 * 128, c1 - c0],
                         [128, 8],
                         [1, 32]])
                    nc.sync.dma_start(dst, src)

        psP_cm.__exit__(None, None, None)
        pin_cm.__exit__(None, None, None)

        # ---------------- flash self-attention ----------------
        # attn@V and the softmax denominator accumulate together in PSUM
        # across the 32 key chunks: per head the av lhsT is 33 wide (32
        # value channels + a ones column), so oasc_g2 row 32/96 ends up
        # holding sum(exp) for the pair's two heads (64-col bands 0/64).
        psacc_cm = tc.tile_pool(name="psacc", bufs=1, space="PSUM")
        psacc = psacc_cm.__enter__()
        oasc0 = psacc.tile([128, LQC], F32, tag="oasc0")
        oasc1 = psacc.tile([128, LQC], F32, tag="oasc1")
        oasc2 = psacc.tile([128, LQC], F32, tag="oasc2")
        oasc3 = psacc.tile([128, LQC], F32, tag="oasc3")
        oasc = [oasc0, oasc1, oasc2, oasc3]
        psA_cm = tc.tile_pool(name="psA", bufs=2, space="PSUM")
        psA = psA_cm.__enter__()

        for ck in range(NKC):
            for g2 in range(4):
                ps = psA.tile([128, 1024], F32, tag="pscore")
                for j in range(2):
                    h = 2 * g2 + j
                    m, hh = h // 4, h % 4
                    nc.tensor.matmul(ps[:, j * 512:(j + 1) * 512],
                                     kpT[32 * hh:32 * hh + 32, m, ck * 128:(ck + 1) * 128],
                                     qpT[32 * hh:32 * hh + 32, m, :],
                                     start=True, stop=True, tile_position=(32 * hh, 0))
                eb = ebp.tile([128, 1024], BF16, tag="eb")
                nc.scalar.activation(eb[:], ps[:], AF.Exp)
                first, last = (ck == 0), (ck == NKC - 1)
                for j in range(2):
                    h = 2 * g2 + j
                    nc.tensor.matmul(oasc[g2][64 * j:64 * j + 33, :],
                                     vp[:, ck, h],
                                     eb[:, j * 512:(j + 1) * 512],
                                     start=first, stop=last,
                                     tile_position=(0, 64 * j),
                                     skip_group_check=True)

        # finalize: broadcast sumexp rows to the 32-row head bands, divide
        srow = fls.tile([1, 8, LQC], BF16, tag="srow")
        for g2 in range(4):
            nc.vector.tensor_copy(srow[:, 2 * g2], oasc[g2][32:33, :])
            nc.vector.tensor_copy(srow[:, 2 * g2 + 1], oasc[g2][96:97, :])
        psA_cm.__exit__(None, None, None)
        psbc_cm = tc.tile_pool(name="psbc", bufs=1, space="PSUM")
        psbc = psbc_cm.__enter__()
        pbt0 = psbc.tile([128, LQC], F32, tag="psbc0")
        pbt1 = psbc.tile([128, LQC], F32, tag="psbc1")
        pbt = [pbt0, pbt1]
        ones1x32 = cst.tile([1, 32], BF16, tag="ones1x32")
        nc.gpsimd.memset(ones1x32[:], 1.0)
        for h in range(8):
            m, a = h // 4, h % 4
            nc.tensor.matmul(pbt[m][32 * a:32 * a + 32, :], ones1x32[:],
                             srow[:, h], start=True, stop=True,
                             tile_position=(0, 32 * a), skip_group_check=True)
        rsb = fls.tile([128, 2, LQC], F32, tag="rsb")
        ocat = res.tile([128, 2, LQC], BF16, tag="ocat")
        for m in range(2):
            nc.vector.reciprocal(rsb[:, m], pbt[m][:])
        for h in range(8):
            g2, j = h // 2, h % 2
            m, a = h // 4, h % 4
            nc.vector.tensor_mul(ocat[32 * a:32 * a + 32, m],
                                 oasc[g2][64 * j:64 * j + 32, :],
                                 rsb[32 * a:32 * a + 32, m])

        psbc_cm.__exit__(None, None, None)
        psacc_cm.__exit__(None, None, None)
        fls_cm.__exit__(None, None, None)
        post = ctx.enter_context(tc.tile_pool(name="post", bufs=1))
        post2 = ctx.enter_context(tc.tile_pool(name="post2", bufs=3))
        psB = ctx.enter_context(tc.tile_pool(name="psB", bufs=2, space="PSUM"))
        psD = ctx.enter_context(tc.tile_pool(name="psD", bufs=1, space="PSUM"))
        psS = ctx.enter_context(tc.tile_pool(name="psS", bufs=2, space="PSUM"))
        # level-2 table build first: tiny, and its gathers run first
        emit_build(2, psS)

        onesf = cst.tile([128, 1], F32, tag="onesf")
        nc.gpsimd.memset(onesf[:], 1.0)
        one1 = cst.tile([1, 1], F32, tag="one1")
        nc.gpsimd.memset(one1[:], 1.0)
        ones1x128 = cst.tile([1, 128], F32, tag="ones1x128")
        nc.gpsimd.memset(ones1x128[:], 1.0)

        def lnorm(pre, dst_f32, dst_bf, which, W=LQC):
            pm = psB.tile([128, W], F32, tag="ps512")
            for k in range(2):
                nc.tensor.matmul(pm[0:1, :], onesf[:], pre[:, k], start=(k == 0), stop=(k == 1))
            pm2 = psB.tile([128, W], F32, tag="ps512")
            for k in range(2):
                sq = post.tile([128, W], F32, tag="sq")
                nc.vector.tensor_mul(sq[:], pre[:, k], pre[:, k])
                nc.tensor.matmul(pm2[0:1, :], onesf[:], sq[:], start=(k == 0), stop=(k == 1))
            mean = post.tile([1, W], F32, tag="mean")
            nc.scalar.mul(mean[:], pm[0:1, :], 1.0 / D)
            var = post.tile([1, W], F32, tag="var")
            nc.scalar.mul(var[:], pm2[0:1, :], 1.0 / D)
            msq = post.tile([1, W], F32, tag="lv")
            nc.vector.tensor_mul(msq[:], mean[:], mean[:])
            nc.vector.tensor_sub(var[:], var[:], msq[:])
            nc.vector.tensor_scalar(var[:], var[:], EPS, None, ALU.add)
            lv = post.tile([1, W], F32, tag="lv")
            nc.scalar.activation(lv[:], var[:], AF.Ln)
            rstd = post.tile([1, W], F32, tag="rstd")
            nc.scalar.activation(rstd[:], lv[:], AF.Exp, scale=-0.5)
            m2 = post.tile([1, W], F32, tag="m2")
            nc.vector.tensor_mul(m2[:], mean[:], rstd[:])
            pb = psD.tile([128, 2 * W], F32, tag="pbc")
            nc.tensor.matmul(pb[:, 0:W], ones1x128[:], rstd[:], start=True, stop=True)
            nc.tensor.matmul(pb[:, W:2 * W], ones1x128[:], m2[:], start=True, stop=True)
            bca = post.tile([128, 2 * W], F32, tag="bca")
            nc.vector.tensor_copy(bca[:], pb[:])
            for k in range(2):
                tn = post.tile([128, W], F32, tag="tn")
                nc.vector.tensor_mul(tn[:], pre[:, k], bca[:, 0:W])
                nc.vector.tensor_sub(tn[:], tn[:], bca[:, W:2 * W])
                nc.vector.tensor_scalar(
                    dst_f32[:, k], tn[:],
                    t_in["lnp"][:, k, 2 * which:2 * which + 1],
                    t_in["lnp"][:, k, 2 * which + 1:2 * which + 2],
                    ALU.mult, ALU.add)
                if dst_bf is not None:
                    nc.vector.tensor_copy(dst_bf[:, k], dst_f32[:, k])

        x1 = post.tile([128, 2, LQC], F32, tag="x1")
        pre1 = post.tile([128, 2, LQC], F32, tag="pre")
        for m in range(2):
            po = psB.tile([128, LQC], F32, tag="ps512")
            for k in range(2):
                nc.tensor.matmul(po[:], t_in["woT"][:, k, m * 128:(m + 1) * 128],
                                 ocat[:, k], start=(k == 0), stop=(k == 1))
            nc.vector.tensor_add(pre1[:, m], t_in["bqcT"][:, m], po[:])
        lnorm(pre1, x1, None, 0)

        # ---------------- deformable ----------------
        q2 = post.tile([128, 2, LQC], BF16, tag="q2")
        for k in range(2):
            nc.vector.tensor_add(q2[:, k], x1[:, k], t_in["poscT"][:, k])

        offq = post.tile([128, NQC, 192], F32, tag="offq")
        awq = post.tile([128, NQC, 96], F32, tag="awq")
        for qc in range(NQC):
            pof = psB.tile([128, 512], F32, tag="ps512")
            for k in range(2):
                nc.tensor.matmul(pof[:, 0:192], q2[:, k, qc * 128:(qc + 1) * 128],
                                 t_in["offwT"][:, k], start=(k == 0), stop=False)
            nc.tensor.matmul(pof[:, 0:192], ones1x128[:], t_in["offb"][:],
                             start=False, stop=True)
            nc.vector.tensor_copy(offq[:, qc], pof[:, 0:192])
            paw = psB.tile([128, 512], F32, tag="ps512")
            for k in range(2):
                nc.tensor.matmul(paw[:, 0:96], q2[:, k, qc * 128:(qc + 1) * 128],
                                 t_in["attnwT"][:, k], start=(k == 0), stop=(k == 1))
            eaw = post.tile([128, 96], F32, tag="eaw")
            nc.scalar.activation(eaw[:], paw[:, 0:96], AF.Exp)
            sm = post.tile([128, 8], F32, tag="sm")
            nc.vector.tensor_reduce(sm[:], eaw[:].rearrange("p (h s) -> p h s", h=8),
                                    mybir.AxisListType.X, ALU.add)
            rsm = post.tile([128, 8], F32, tag="rsm")
            nc.vector.reciprocal(rsm[:], sm[:])
            nc.vector.tensor_mul(awq[:, qc].rearrange("p (h s) -> p h s", h=8),
                                 eaw[:].rearrange("p (h s) -> p h s", h=8),
                                 rsm[:].unsqueeze(2).broadcast_to([128, 8, 12]))

        ocaq = post.tile([128, NQC, D], F32, tag="ocaq")
        for qc in range(NQC):
            nc.gpsimd.memset(ocaq[:, qc], 0.0)

        qnames = ["qPoolDynamic", "qPoolDynamic1", "qPoolDynamic2",
                  "qPoolDynamic3"]
        # pass 1: compute gather indices + corner weights, batched over all
        # NQC query chunks per level (4x fewer DVE instructions).
        idx_t = {}
        cw_t = {}
        NJ = NQC * 32

        def emit_pass1(l):
            hl, wl = SHAPES[l]
            w2 = W2S[l]
            if True:
                off6 = offq[:].rearrange("p q (h l k two) -> p q h l k two",
                                         h=8, l=3, k=4)
                xo = off6[:, :, :, l, :, 0]
                yo = off6[:, :, :, l, :, 1]
                refx = post.tile([128, NQC, 1], F32, tag="refx")
                nc.vector.tensor_scalar(refx[:], t_in["refq"][:, :, 2 * l:2 * l + 1],
                                        float(wl), -0.5, ALU.mult, ALU.add)
                refy = post.tile([128, NQC, 1], F32, tag="refy")
                nc.vector.tensor_scalar(refy[:], t_in["refq"][:, :, 2 * l + 1:2 * l + 2],
                                        float(hl), -0.5, ALU.mult, ALU.add)
                xs = post.tile([128, NQC, 32], F32, tag="xs")
                nc.vector.tensor_tensor(
                    xs[:].rearrange("p q (h c) -> p q h c", h=8), xo,
                    refx[:].unsqueeze(3).broadcast_to([128, NQC, 8, 4]), ALU.add)
                ys = post.tile([128, NQC, 32], F32, tag="ys")
                nc.vector.tensor_tensor(
                    ys[:].rearrange("p q (h c) -> p q h c", h=8), yo,
                    refy[:].unsqueeze(3).broadcast_to([128, NQC, 8, 4]), ALU.add)

                def floorpair(src, tag):
                    # robust floor: t = int(src+16); tf = float(t) - 16;
                    # if tf > src: tf -= 1   (works for trunc or round)
                    ti = post.tile([128, NQC, 32], I32, tag=tag + "i")
                    tsh = post.tile([128, NQC, 32], F32, tag=tag + "sh")
                    nc.vector.tensor_scalar(tsh[:], src[:], 16.0, None, ALU.add)
                    nc.vector.tensor_copy(ti[:], tsh[:])
                    tf = post.tile([128, NQC, 32], F32, tag=tag + "f")
                    nc.vector.tensor_copy(tf[:], ti[:])
                    nc.vector.tensor_scalar(tf[:], tf[:], -16.0, None, ALU.add)
                    gt = post.tile([128, NQC, 32], F32, tag=tag + "g")
                    nc.vector.tensor_tensor(gt[:], tf[:], src[:], ALU.is_gt)
                    nc.vector.tensor_sub(tf[:], tf[:], gt[:])
                    w = post.tile([128, NQC, 32], F32, tag=tag + "w")
                    nc.vector.tensor_sub(w[:], src[:], tf[:])
                    return tf, w

                x0f, wx = floorpair(xs, "fx")
                y0f, wy = floorpair(ys, "fy")

                def vmask(base_f, hi, tag):
                    v0 = post.tile([128, NQC, 32], F32, tag=tag + "v0")
                    nc.vector.tensor_scalar(v0[:], base_f[:], 0.0, None, ALU.is_ge)
                    t = post.tile([128, NQC, 32], F32, tag=tag + "t")
                    nc.vector.tensor_scalar(t[:], base_f[:], float(hi - 1), None, ALU.is_le)
                    nc.vector.tensor_mul(v0[:], v0[:], t[:])
                    v1 = post.tile([128, NQC, 32], F32, tag=tag + "v1")
                    nc.vector.tensor_scalar(v1[:], base_f[:], -1.0, None, ALU.is_ge)
                    nc.vector.tensor_scalar(t[:], base_f[:], float(hi - 2), None, ALU.is_le)
                    nc.vector.tensor_mul(v1[:], v1[:], t[:])
                    return v0, v1

                vx0, vx1 = vmask(x0f, wl, "vx")
                vy0, vy1 = vmask(y0f, hl, "vy")

                awt = post.tile([128, NQC, 32], F32, tag="awt")
                nc.vector.tensor_copy(
                    awt[:].rearrange("p q (h c) -> p q h c", h=8),
                    awq[:].rearrange("p q (h s) -> p q h s", h=8)[:, :, :, l * 4:(l + 1) * 4])

                wx0 = post.tile([128, NQC, 32], F32, tag="wx0")
                nc.vector.tensor_scalar(wx0[:], wx[:], -1.0, 1.0, ALU.mult, ALU.add)
                nc.vector.tensor_mul(wx0[:], wx0[:], vx0[:])
                wx1 = post.tile([128, NQC, 32], F32, tag="wx1")
                nc.vector.tensor_mul(wx1[:], wx[:], vx1[:])
                wy0 = post.tile([128, NQC, 32], F32, tag="wy0")
                nc.vector.tensor_scalar(wy0[:], wy[:], -1.0, 1.0, ALU.mult, ALU.add)
                nc.vector.tensor_mul(wy0[:], wy0[:], vy0[:])
                nc.vector.tensor_mul(wy0[:], wy0[:], awt[:])
                wy1 = post.tile([128, NQC, 32], F32, tag="wy1")
                nc.vector.tensor_mul(wy1[:], wy[:], vy1[:])
                nc.vector.tensor_mul(wy1[:], wy1[:], awt[:])

                cw = post.tile([128, NQC, 32, 4], F32, tag=f"cw{l}")
                cw_t[l] = cw
                nc.vector.tensor_mul(cw[:, :, :, 0], wy0[:], wx0[:])
                nc.vector.tensor_mul(cw[:, :, :, 1], wy0[:], wx1[:])
                nc.vector.tensor_mul(cw[:, :, :, 2], wy1[:], wx0[:])
                nc.vector.tensor_mul(cw[:, :, :, 3], wy1[:], wx1[:])

                # float index (quad-major rows):
                #   (clip(y0+1,0,hl)*w2 + clip(x0+1,0,wl))*8 + h  (cb = h)
                xcf = post.tile([128, NQC, 32], F32, tag="xcf")
                nc.vector.tensor_scalar(xcf[:], x0f[:], 1.0, 0.0, ALU.add, ALU.max)
                nc.vector.tensor_scalar(xcf[:], xcf[:], float(wl), 8.0, ALU.min, ALU.mult)
                ycf = post.tile([128, NQC, 32], F32, tag="ycf")
                nc.vector.tensor_scalar(ycf[:], y0f[:], 1.0, 0.0, ALU.add, ALU.max)
                nc.vector.tensor_scalar(ycf[:], ycf[:], float(hl), None, ALU.min)
                idxf = post.tile([128, NQC, 32], F32, tag="idxf")
                nc.vector.tensor_scalar(idxf[:], ycf[:], float(w2 * 8), None, ALU.mult)
                nc.vector.tensor_add(idxf[:], idxf[:], xcf[:])
                nc.vector.tensor_add(
                    idxf[:], idxf[:],
                    t_in[f"cb{l}"][:].unsqueeze(1).broadcast_to([128, NQC, 32]))
                idx = post.tile([128, NQC, 32], I32, tag=f"idx{l}")
                idx_t[l] = idx
                nc.vector.tensor_copy(idx[:], idxf[:])

        # pass 2: per level (2 -> 1 -> 0): table build (PE shifts + scalar
        # copies, so the DVE stream stays free for interpolation), then the
        # level's gathers + interpolation. Emission order: a gather waits on
        # every earlier-emitted qt write (per-tensor DRAM deps), so each
        # level's build is emitted right before its own gathers. After a qc's
        # last chunk (level 0), its transpose + ca-out projection run under
        # the remaining gathers.
        ocab = post.tile([128, NQC, D], BF16, tag="ocab")
        ocaT = post.tile([128, 2, LQC], BF16, tag="ocaT")
        pre2 = post.tile([128, 2, LQC], F32, tag="pre")
        gq_t = {}

        _gq_ctr = [0]

        def emit_gather(l, qc):
            # one batched indirect DMA: 4096 offsets (32 rows x 128
            # partitions) in a single instruction -- the SWDGE per-call
            # fixed cost is paid once instead of 32 times.
            gq = post2.tile([128, 32, 128], BF16, tag="gq")
            gq_t[(l, qc)] = gq
            if BATCH_GATHER:
                gi_inst = nc.gpsimd.indirect_dma_start(
                    gq[:], None, qta_l[l],
                    IndirectOffsetOnAxis(ap=idx_t[l][:, qc], axis=0))
                gi_inst.ins.queue = qnames[_gq_ctr[0] % 4]
                _gq_ctr[0] += 1
            else:
                idx = idx_t[l]
                for j in range(32):
                    gi_inst = nc.gpsimd.indirect_dma_start(
                        gq[:, j, :], None, qta_l[l],
                        IndirectOffsetOnAxis(ap=idx[:, qc, j:j + 1], axis=0))
                    gi_inst.ins.queue = qnames[j % 4]

        def emit_interp(l, qc):
            if True:
                cw = cw_t[l][:, qc]
                gq = gq_t[(l, qc)]
                tmp = post.tile([128, 32, 4, 32], BF16, tag="tmpc")
                nc.vector.tensor_mul(
                    tmp[:], gq[:].rearrange("p j (s c) -> p j s c", s=4),
                    cw.unsqueeze(3).broadcast_to([128, 32, 4, 32]))
                red = post.tile([128, 8, 32], F32, tag="red")
                nc.vector.tensor_reduce(
                    red[:],
                    tmp[:].rearrange("p (h pp) s c -> p h c pp s", h=8),
                    mybir.AxisListType.XY, ALU.add)
                nc.vector.tensor_add(
                    ocaq[:, qc].rearrange("p (h c) -> p h c", h=8),
                    ocaq[:, qc].rearrange("p (h c) -> p h c", h=8), red[:])
                if l == 0:
                    # qc complete: transpose + ca-out proj columns now
                    nc.vector.tensor_copy(ocab[:, qc], ocaq[:, qc])
                    for dc in range(2):
                        pt = psD.tile([128, 128], BF16, tag="ptc")
                        nc.tensor.transpose(pt[:], ocab[:, qc, dc * 128:(dc + 1) * 128],
                                            t_in["idt"][:])
                        nc.vector.tensor_copy(ocaT[:, dc, qc * 128:(qc + 1) * 128],
                                              pt[:])
                    qs = slice(qc * 128, (qc + 1) * 128)
                    for m in range(2):
                        pc = psB.tile([128, 512], F32, tag="ps512")
                        for k in range(2):
                            nc.tensor.matmul(pc[:, 0:128],
                                             t_in["cawT"][:, k, m * 128:(m + 1) * 128],
                                             ocaT[:, k, qs],
                                             start=(k == 0), stop=(k == 1))
                        nc.vector.tensor_add(pre2[:, m, qs], x1[:, m, qs], pc[:, 0:128])
                    emit_tail(qc)

        # per-chunk tail: lnorm2 + FFN + lnorm3 + store for one 128-query
        # chunk, emitted as soon as its last interp + ca-out are in -- the
        # final chunk's tail is all that remains after the gather stream.
        x2 = post.tile([128, 2, LQC], F32, tag="x2")
        x2b = post.tile([128, 2, LQC], BF16, tag="x2b")
        h1 = post.tile([128, 4, LQC], BF16, tag="h1")
        pre3 = post.tile([128, 2, LQC], F32, tag="pre3")
        x3 = post.tile([128, 2, LQC], F32, tag="x3")

        def emit_tail(qc):
            qs = slice(qc * 128, (qc + 1) * 128)
            lnorm(pre2[:, :, qs], x2[:, :, qs], x2b[:, :, qs], 1, W=128)
            for m in range(4):
                ph = psB.tile([128, 128], F32, tag="ps512")
                for k in range(2):
                    nc.tensor.matmul(ph[:], t_in["w1T"][:, k, m * 128:(m + 1) * 128],
                                     x2b[:, k, qs], start=(k == 0), stop=(k == 1))
                nc.vector.tensor_scalar(h1[:, m, qs], ph[:], 0.0, None, ALU.max)
            for m in range(2):
                pf = psB.tile([128, 128], F32, tag="ps512")
                for k in range(4):
                    nc.tensor.matmul(pf[:], t_in["w2T"][:, k, m * 128:(m + 1) * 128],
                                     h1[:, k, qs], start=(k == 0), stop=(k == 3))
                nc.vector.tensor_add(pre3[:, m, qs], x2[:, m, qs], pf[:])
            lnorm(pre3[:, :, qs], x3[:, :, qs], None, 2, W=128)
            for k in range(2):
                nc.sync.dma_start(outT.rearrange("(c p) n -> p c n", p=128)[:, k, qs],
                                  x3[:, k, qs])

        # schedule: all pass-1 index math up front (before any gather, so a
        # gather's conservative wait on earlier-emitted vector ops is already
        # satisfied), then table builds for levels 1/0 (level 2 was built
        # early, during flash), then the gather stream, CHUNK-major so each
        # 128-query chunk finishes (interp + ca-out + lnorm2/FFN/store) while
        # later chunks still gather; interps trail by 1 for gq buffering.
        CHUNKS = [(l_, qc_) for qc_ in range(NQC) for l_ in (2, 1, 0)]
        emit_pass1(2)
        emit_pass1(1)
        emit_pass1(0)
        emit_build(1, psS)
        emit_build(0, psS)
        for i_, (l_, qc_) in enumerate(CHUNKS):
            emit_gather(l_, qc_)
            if i_ >= 1:
                emit_interp(*CHUNKS[i_ - 1])
        emit_interp(*CHUNKS[-1])

    if split_waits:
        _split_multi_waits(nc)
    return nc


_NC_CACHE = {}
TRACE = False


def kernel(**inputs):
    bq = np.asarray(inputs["bev_queries"])[0]
    img = np.asarray(inputs["img_feats_flat"])[0]
    ref = np.asarray(inputs["ref_points"])[0]
    pos = np.asarray(inputs["bev_pos"])[0]
    miw = np.asarray(inputs["mha_in_w"]); mow = np.asarray(inputs["mha_out_w"])
    offw = np.asarray(inputs["off_w"]); offb = np.asarray(inputs["off_b"])
    attw = np.asarray(inputs["attn_w"])
    valw = np.asarray(inputs["val_w"]); caw = np.asarray(inputs["ca_out_w"])
    w1 = np.asarray(inputs["ffn_w1"]); w2 = np.asarray(inputs["ffn_w2"])

    bf = ml_dtypes.bfloat16
    common = {
        "bqT_bf": np.ascontiguousarray(bq.T).astype(bf),
        "posT_bf": np.ascontiguousarray(pos.T).astype(bf),
        "imgT_bf": np.ascontiguousarray(img.T).astype(bf),
        "wqT": np.ascontiguousarray((miw[0:256] / np.sqrt(32.0)).T).astype(bf),
        "wkT": np.ascontiguousarray(miw[256:512].T).astype(bf),
        "wvT": np.ascontiguousarray(miw[512:768].T).astype(bf),
        "woT": np.ascontiguousarray(mow.T).astype(bf),
        "offwT": np.ascontiguousarray(offw.T).astype(bf),
        "offb": offb.reshape(1, 192).astype(np.float32),
        "attnwT": np.ascontiguousarray(attw.T).astype(bf),
        "valwT": np.ascontiguousarray(valw.T).astype(bf),
        "cawT": np.ascontiguousarray(caw.T).astype(bf),
        "w1T": np.ascontiguousarray(w1.T).astype(bf),
        "w2T": np.ascontiguousarray(w2.T).astype(bf),
        "idt": np.eye(128).astype(bf),
    }
    pshift = np.zeros((128, 14, 128), np.float32)
    for si, sh in enumerate([1, 16, 17, 32, 33, 64, 65]):
        for m_ in range(128 - sh):
            pshift[m_ + sh, 2 * si, m_] = 1.0      # main: out[m] = in[m+sh]
        for m_ in range(128 - sh, 128):
            pshift[m_ + sh - 128, 2 * si + 1, m_] = 1.0  # wrap
    common["pshift"] = pshift.reshape(128, 14 * 128).astype(bf)
    lnp = np.zeros((128, 2, 6), np.float32)
    prs = [(inputs["norm1_g"], inputs["norm1_b"]),
           (inputs["norm2_g"], inputs["norm2_b"]),
           (inputs["ffn_g"], inputs["ffn_bb"])]
    for k in range(2):
        sl = slice(k * 128, (k + 1) * 128)
        for i, (g, b) in enumerate(prs):
            lnp[:, k, 2 * i] = np.asarray(g)[sl]
            lnp[:, k, 2 * i + 1] = np.asarray(b)[sl]
    common["lnp"] = lnp
    for l in range(3):
        cb = np.zeros((128, 32), np.float32)
        for h in range(8):
            cb[:, h * 4:(h + 1) * 4] = h  # quad-major rows: idx = quad*8 + h
        common[f"cb{l}"] = cb

    if "nc" not in _NC_CACHE:
        _NC_CACHE["nc"] = build_kernel()
    nc = _NC_CACHE["nc"]

    in_maps = []
    for c in range(8):
        sl = slice(c * LQC, (c + 1) * LQC)
        m = dict(common)
        m["bqcT"] = np.ascontiguousarray(bq[sl].T).astype(np.float32)
        m["poscT"] = np.ascontiguousarray(pos[sl].T).astype(np.float32)
        refc = ref[sl].reshape(NQC, 128, 6)
        m["refq"] = np.ascontiguousarray(refc.transpose(1, 0, 2)).astype(np.float32)
        in_maps.append(m)

    res = run_bass_kernel_spmd(nc, in_maps, list(range(8)), trace=TRACE)
    _NC_CACHE["last_res"] = res
    out = np.zeros((1, LQ, D), np.float32)
    for c in range(8):
        out[0, c * LQC:(c + 1) * LQC, :] = res.results[c]["outT"].T
    return out

